# revision 6
# baseline (speedup 1.0000x reference)
"""MoE MLP (top-2 routing, 8 experts) on 8 Trainium2 NeuronCores.

Strategy (expert-parallel, per the sharding hint): each core owns one
expert's weights. The router (a [8,1024] matmul + softmax + top-2 —
0.05% of total FLOPs) runs on the host, which doubles as the dispatch
step: tokens are gathered per selected expert and shipped to that
expert's core, replacing the all-to-all. Each core runs a fused
gelu-MLP Bass kernel over its routed tokens:

    yT = w ⊙ (W_out^T @ gelu(W_in^T @ xT + b_in) + b_out)

in a transposed layout (tokens along the free axis) so both matmuls
keep the *weights* stationary on the PE array and no on-chip
transposes are needed anywhere. BOTH weight matrices live resident in
SBUF — W_in stripes load once during chunk 0's phase A (alternating
across the two HWDGE queues, stripe 1 split so every stripe lands
>=3us before its consumption slot), W_out rides behind them and is
fully resident ~15us before phase B first needs it. After ~70us the
only DMA traffic is the token stream in and the result stream out.
The host scatter-adds the per-expert results back into the full
[B,S,D] output.

Matmuls run in fp16 (same PE throughput as bf16 — 4x fp32 — but 8x
finer mantissa; measured end-to-end error vs the fp32 reference is
~4e-4 scale-relative).

Startup discipline (HW-measured): the HWDGE queues deliver no data for
the first ~8-11us (NEFF pre-roll), and the HAM activity manager
restarts its ~10us half-speed ramp whenever the PE idles more than a
couple of us — so the PE must start junk work early (memset on the
fast-starting DVE, not gpsimd) and then never starve. Monolithic
x0/W_in-stripe-0 loads split across both queues gate the first real
matmul at ~19us; fine-grained early starts measurably LOSE (the
framework coalesces the DMA-completion waits, and the stop-start PE
activity re-triggers the HAM clamp for 40+us).
"""

import contextlib
import ctypes
import os
import sys
import types
from contextlib import ExitStack

import numpy as np

import concourse.bass as bass
import concourse.mybir as mybir
import concourse.tile as tile
from concourse import bacc
from concourse.bass_utils import run_bass_kernel_spmd


def _install_ntff_hook():
    """Provide antenv.axon_hooks (absent in this image) so BASS_TRACE=1
    can capture NTFF profiles through the axon PJRT .so. No-op if the
    module already exists or the .so/symbols are unavailable."""
    try:
        from antenv.axon_hooks import get_axon_ntff_profile_hook  # noqa: F401
        return
    except ImportError:
        pass
    so_path = "/opt/axon/libaxon_pjrt.so"
    if not os.path.exists(so_path):
        return
    try:
        lib = ctypes.CDLL(so_path)
    except OSError:
        return
    if not hasattr(lib, "axon_start_nrt_profile"):
        return
    lib.axon_start_nrt_profile.argtypes = [
        ctypes.POINTER(ctypes.c_int64), ctypes.c_size_t]
    lib.axon_start_nrt_profile.restype = ctypes.c_int64
    lib.axon_stop_nrt_profile.argtypes = [ctypes.c_char_p]
    lib.axon_stop_nrt_profile.restype = ctypes.c_int64

    @contextlib.contextmanager
    def _hook(output_dir, device_ids):
        import jax
        jax.devices()  # force PJRT init so the .so's client exists
        if device_ids:
            ids = (ctypes.c_int64 * len(device_ids))(*device_ids)
            rc = lib.axon_start_nrt_profile(ids, len(device_ids))
        else:
            rc = lib.axon_start_nrt_profile(None, 0)
        if rc != 0:
            raise RuntimeError(f"axon_start_nrt_profile rc={rc}")
        try:
            yield
        finally:
            n = lib.axon_stop_nrt_profile(str(output_dir).encode())
            print(f"ntff profile: {n} file(s) -> {output_dir}", file=sys.stderr)

    import antenv
    mod = types.ModuleType("antenv.axon_hooks")
    mod.get_axon_ntff_profile_hook = lambda: _hook
    mod.set_axon_ntff_profile_hook = lambda h: None
    sys.modules["antenv.axon_hooks"] = mod
    antenv.axon_hooks = mod

B, S, D, F, E = 4, 2048, 1024, 4096, 8
T = B * S
TOP_K = 2
NCORES = 8
P = 128
ND, NF = D // P, F // P  # 8, 32
NFO = F // 512           # 8 (512-wide stripes of F)

# test.py pokes these for profiling info
LAST_RESULT = None

_cache = {}


def _chunk_list(C):
    """Token chunks (PSUM free-dim <= 512, multiples of 128).

    Chunks below 256 run LDWEIGHTS-bound on the PE (weight load ~60ns
    vs a 53ns N=128 matmul), so a short tail is split off the previous
    512 chunk into two >=256 pieces instead.
    """
    chunks = [512] * (C // 512)
    rem = C % 512
    if rem:
        if rem < 256 and chunks:
            total = 512 + rem
            a = ((total // 2 + 127) // 128) * 128
            chunks[-1] = a
            chunks.append(total - a)
        else:
            chunks.append(rem)
    return chunks


def _build_bass(C):
    dt = mybir.dt
    io_dt = dt.float16
    nc = bacc.Bacc("TRN2", target_bir_lowering=False, debug=False)

    xT = nc.dram_tensor("xT", [D, C], io_dt, kind="ExternalInput")
    # W_in host-packed: stripe-major [p][fo][dn][512] so each 512-wide
    # F-stripe is one contiguous 8KB-per-partition DMA.
    win = nc.dram_tensor("win", [P, NFO * ND * 512], io_dt, kind="ExternalInput")
    wout = nc.dram_tensor("wout", [F, D], io_dt, kind="ExternalInput")
    # b_in/b_out host-packed to [partition, idx] (contiguous rows; the
    # naive (fo fi) gather is 4096 4-byte descriptors on the SWDGE).
    bin_ = nc.dram_tensor("bin", [P, NF], dt.float32, kind="ExternalInput")
    bout = nc.dram_tensor("bout", [P, ND], dt.float32, kind="ExternalInput")
    wcomb = nc.dram_tensor("wcomb", [P, C], dt.float32, kind="ExternalInput")
    yT = nc.dram_tensor("yT", [D, C], dt.float32, kind="ExternalOutput")

    xT_r = xT.ap().rearrange("(dn p) c -> p dn c", p=P)
    win_r = win.ap().rearrange("p (fo dn f) -> p fo dn f", fo=NFO, f=512)
    wout_r = wout.ap().rearrange("(fn p) d -> p fn d", p=P)
    yT_r = yT.ap().rearrange("(dn p) c -> p dn c", p=P)

    chunks = _chunk_list(C)
    ck0 = chunks[0]

    with tile.TileContext(nc) as tc, ExitStack() as ctx:
        consts = ctx.enter_context(tc.tile_pool(name="consts", bufs=1))
        xpool = ctx.enter_context(tc.tile_pool(name="x", bufs=2))
        wrespool = ctx.enter_context(tc.tile_pool(name="wres", bufs=1))
        woutpool = ctx.enter_context(tc.tile_pool(name="wout", bufs=1))
        hpool = ctx.enter_context(tc.tile_pool(name="h", bufs=1))
        ypool = ctx.enter_context(tc.tile_pool(name="y", bufs=3))
        psum_h = ctx.enter_context(tc.tile_pool(name="ph", bufs=4, space="PSUM"))
        psum_y = ctx.enter_context(tc.tile_pool(name="py", bufs=2, space="PSUM"))

        # critical path for the very first matmul: x chunk 0 + W_in
        # stripe 0 go FIRST, each split across BOTH HWDGE queues (Sync +
        # Act) — a single dma_start runs ~150 GB/s, so two in parallel
        # roughly halve the time to first matmul.
        x0_t = xpool.tile([P, ND, ck0], io_dt, tag="x")
        nc.sync.dma_start(x0_t[:, :4, :], xT_r[:, :4, 0:ck0])
        nc.scalar.dma_start(x0_t[:, 4:, :], xT_r[:, 4:, 0:ck0])
        win_t = [wrespool.tile([P, ND, 512], io_dt, name=f"wfo{fo}")
                 for fo in range(NFO)]
        nc.sync.dma_start(win_t[0][:, :4, :], win_r[:, 0, 0:4, :])
        nc.scalar.dma_start(win_t[0][:, 4:, :], win_r[:, 0, 4:, :])

        # b_in is needed by the first gelu (~25us); host-packed rows on
        # the SWDGE queue land ~15us.
        bin_t = consts.tile([P, NF], dt.float32)
        nc.gpsimd.dma_start(bin_t[:], bin_.ap())

        # PE HAM warm-up: ~4us of junk matmuls on a scratch tile while
        # the x0/win0 DMAs are in flight, so the HAM activity ramp
        # (3.4us full + 6.8us half speed from first PE activity) is
        # burned through before real matmuls start. memset on the DVE —
        # it is live ~3us in, while gpsimd takes ~12us to start.
        wu_t = consts.tile([P, P], io_dt)
        nc.vector.memset(wu_t[:], 0.0)
        wu_ps = ctx.enter_context(tc.tile_pool(name="wups", bufs=1, space="PSUM"))
        wu_p = wu_ps.tile([P, 64], dt.float32)
        for _ in range(60):
            nc.tensor.matmul(wu_p[:], wu_t[:], wu_t[:, :64], start=True, stop=True)

        # Remaining W_in stripes alternate queues (stripe 1 split across
        # both so it lands well before its ~26us consumption slot); each
        # stripe arrives ~6.7us apart per queue against a 13.6us/pair
        # consumption rate, so the margin grows monotonically. W_out
        # rides behind on both queues — fully resident ~71us, needed
        # ~86us (phase B of chunk 0).
        nc.sync.dma_start(win_t[1][:, :4, :], win_r[:, 1, 0:4, :])
        nc.scalar.dma_start(win_t[1][:, 4:, :], win_r[:, 1, 4:, :])
        for fo in range(2, NFO):
            eng = nc.sync if fo % 2 == 0 else nc.scalar
            eng.dma_start(win_t[fo][:], win_r[:, fo, :, :])

        wout_tiles = []
        for fo in range(NFO):
            t = woutpool.tile([P, 4, D], io_dt, name=f"wout{fo}")
            eng = nc.sync if fo % 2 == 0 else nc.scalar
            eng.dma_start(t[:], wout_r[:, fo * 4:(fo + 1) * 4, :])
            wout_tiles.append(t)

        bout_t = consts.tile([P, ND], dt.float32)
        nc.gpsimd.dma_start(bout_t[:], bout.ap())
        w_t = consts.tile([P, C], dt.float32)
        nc.gpsimd.dma_start(w_t[:], wcomb.ap())

        # ---- main loop ------------------------------------------------
        off = 0
        for ci, ck in enumerate(chunks):
            csl = slice(off, off + ck)
            last = ci == len(chunks) - 1
            if ci == 0:
                x_t = x0_t
            else:
                x_t = xpool.tile([P, ND, ck], io_dt, tag="x")
                nc.scalar.dma_start(x_t[:], xT_r[:, :, csl])

            # ---- phase A: h = gelu(W_in^T @ x + b_in), laid out [f, tok]
            h_t = hpool.tile([P, NF, ck], io_dt, tag="h")
            for fo in range(NFO):
                for j in range(4):
                    fc = fo * 4 + j
                    ph = psum_h.tile([P, ck], dt.float32, tag="ph")
                    for dn in range(ND):
                        nc.tensor.matmul(
                            ph[:],
                            win_t[fo][:, dn, j * P:(j + 1) * P],
                            x_t[:, dn, :],
                            start=(dn == 0),
                            stop=(dn == ND - 1),
                        )
                    nc.scalar.activation(
                        h_t[:, fc, :], ph[:],
                        mybir.ActivationFunctionType.Gelu,
                        bias=bin_t[:, fc:fc + 1],
                    )

            # ---- phase B: y = w * (W_out^T @ h + b_out), laid out [d, tok]
            for dn in range(ND):
                py = psum_y.tile([P, ck], dt.float32, tag="py")
                for fc in range(NF):
                    nc.tensor.matmul(
                        py[:],
                        wout_tiles[fc // 4][:, fc % 4, dn * P:(dn + 1) * P],
                        h_t[:, fc, :],
                        start=(fc == 0),
                        stop=(fc == NF - 1),
                    )
                y_t = ypool.tile([P, ck], dt.float32, tag="y")
                # one DVE op: (psum + b_out) * w — keeps ScalarE on
                # gelu only (no ACT table switching per chunk)
                nc.vector.scalar_tensor_tensor(
                    y_t[:], py[:], bout_t[:, dn:dn + 1], w_t[:, csl],
                    op0=mybir.AluOpType.add, op1=mybir.AluOpType.mult,
                )
                # steady state keeps y on the scalar queue (idle after
                # startup); the final chunk alternates queues so the
                # output drain at end-of-kernel runs in parallel.
                if last:
                    eng = (nc.scalar, nc.sync)[dn % 2]
                else:
                    eng = nc.scalar
                eng.dma_start(yT_r[:, dn, csl], y_t[:])
            off += ck

    nc.compile()
    return nc


def _get_nc(C):
    if C not in _cache:
        _cache[C] = _build_bass(C)
    return _cache[C]


def _route(x, W_router):
    """Host-side router: top-2 selection + renormalized weights (fp64).

    Matches jax.lax.top_k on softmax(logits): softmax is monotone so
    top-2 of logits is identical, with ties broken toward lower index
    (argsort stable on -logits).
    """
    lg = x.astype(np.float64) @ W_router.T.astype(np.float64)
    top2 = np.argsort(-lg, axis=1, kind="stable")[:, :TOP_K]
    l1 = np.take_along_axis(lg, top2[:, 0:1], 1)
    l2 = np.take_along_axis(lg, top2[:, 1:2], 1)
    e2 = np.exp(l2 - l1)
    w1 = (1.0 / (1.0 + e2)).astype(np.float32)
    w2 = (e2 / (1.0 + e2)).astype(np.float32)
    return top2, np.concatenate([w1, w2], axis=1)


def _pack_win(W):
    """[D, F] fp16 -> [P, NFO*ND*512] laid out [p][fo][dn][512]: each
    512-wide F-stripe becomes one contiguous 8KB-per-partition block."""
    A = W.reshape(ND, P, NFO, 512).transpose(1, 2, 0, 3)  # [p, fo, dn, f]
    return np.ascontiguousarray(A).reshape(P, NFO * ND * 512)


def kernel(residual, W_router, W_in, b_in, W_out, b_out):
    global LAST_RESULT

    x = np.ascontiguousarray(np.asarray(residual, dtype=np.float32).reshape(T, D))
    W_in = np.asarray(W_in, dtype=np.float32)
    W_out = np.asarray(W_out, dtype=np.float32)
    b_in = np.asarray(b_in, dtype=np.float32)
    b_out = np.asarray(b_out, dtype=np.float32)

    top2, wts = _route(x, np.asarray(W_router, dtype=np.float32))

    idxs, ws = [], []
    for e in range(E):
        sel0 = top2[:, 0] == e
        sel1 = top2[:, 1] == e
        idx = np.concatenate([np.where(sel0)[0], np.where(sel1)[0]])
        w = np.concatenate([wts[sel0, 0], wts[sel1, 1]])
        idxs.append(idx)
        ws.append(w)

    C = max(len(i) for i in idxs)
    C = ((C + P - 1) // P) * P
    nc = _get_nc(C)

    xt = np.ascontiguousarray(x.T)  # [D, T]
    in_maps = []
    for e in range(E):
        cnt = len(idxs[e])
        xT_e = np.zeros((D, C), dtype=np.float16)
        xT_e[:, :cnt] = xt[:, idxs[e]]
        wc_e = np.zeros((P, C), dtype=np.float32)
        wc_e[:, :cnt] = ws[e][None, :]
        in_maps.append({
            "xT": xT_e,
            "win": _pack_win(np.asarray(W_in[e], dtype=np.float16)),
            "wout": np.ascontiguousarray(W_out[e], dtype=np.float16),
            "bin": np.ascontiguousarray(b_in[e].reshape(NF, P).T),
            "bout": np.ascontiguousarray(b_out[e].reshape(ND, P).T),
            "wcomb": wc_e,
        })

    if os.environ.get("BASS_TRACE"):
        _install_ntff_hook()
    LAST_RESULT = run_bass_kernel_spmd(nc, in_maps, list(range(NCORES)))

    y = np.zeros((T, D), dtype=np.float32)
    for e in range(E):
        cnt = len(idxs[e])
        y[idxs[e]] += LAST_RESULT.results[e]["yT"][:, :cnt].T
    return y.reshape(B, S, D)


# revision 14
# speedup vs baseline: 1.2449x; 1.2449x over previous
"""MoE MLP (top-2 routing, 8 experts) on 8 Trainium2 NeuronCores.

Strategy (expert-parallel, per the sharding hint): each core owns one
expert's weights. The router (a [8,1024] matmul + softmax + top-2 —
0.05% of total FLOPs) runs on the host, which doubles as the dispatch
step: tokens are gathered per selected expert and shipped to that
expert's core, replacing the all-to-all. Each core runs a fused
gelu-MLP Bass kernel over its routed tokens:

    yT = w ⊙ (W_out^T @ gelu(W_in^T @ xT + b_in) + b_out)

in a transposed layout (tokens along the free axis) so both matmuls
keep the *weights* stationary on the PE array and no on-chip
transposes are needed anywhere. BOTH weight matrices are SBUF-resident:
W_in streams in once during chunk 0's phase A (stripe fo rides the
sync queue just ahead of its consumption slot, W_out stripe fo right
behind it — the baseline's proven just-in-time interleave) and is
reused by every later chunk, cutting HBM traffic from ~58MB to ~30MB
per core and removing the W_in-stream stalls on the small tail chunks.
The host scatter-adds the per-expert results back into the full
[B,S,D] output.

Matmuls run in fp16 (same PE throughput as bf16 — 4x fp32 — but 8x
finer mantissa; measured end-to-end error vs the fp32 reference is
~4e-4 scale-relative).

Hard-won scheduling constraints (HW-measured):
- Bulk DMA triggers must stay OFF the scalar/Act queue before the
  first gelu: each engine executes its stream in order and the DMA
  ring is only 4 deep, so queued triggers block the ACTIVATE behind
  them (~20us PE stall via the PSUM-bank WAR chain).
- The HAM activity manager restarts its ~10us half-speed ramp whenever
  the PE idles more than a couple us: the gpsimd-memset warm-up loop
  lands its junk matmuls exactly in the DMA-wait window, and phase A
  must then never starve.
- The tile scheduler reorders upfront DMA emissions; emitting each
  stripe's DMA inside the chunk-0 loop next to its consumer keeps the
  just-in-time order.
"""

import contextlib
import ctypes
import os
import sys
import types
from contextlib import ExitStack

import numpy as np

import concourse.bass as bass
import concourse.mybir as mybir
import concourse.tile as tile
from concourse import bacc
from concourse.bass_utils import run_bass_kernel_spmd


def _install_ntff_hook():
    """Provide antenv.axon_hooks (absent in this image) so BASS_TRACE=1
    can capture NTFF profiles through the axon PJRT .so. No-op if the
    module already exists or the .so/symbols are unavailable."""
    try:
        from antenv.axon_hooks import get_axon_ntff_profile_hook  # noqa: F401
        return
    except ImportError:
        pass
    so_path = "/opt/axon/libaxon_pjrt.so"
    if not os.path.exists(so_path):
        return
    try:
        lib = ctypes.CDLL(so_path)
    except OSError:
        return
    if not hasattr(lib, "axon_start_nrt_profile"):
        return
    lib.axon_start_nrt_profile.argtypes = [
        ctypes.POINTER(ctypes.c_int64), ctypes.c_size_t]
    lib.axon_start_nrt_profile.restype = ctypes.c_int64
    lib.axon_stop_nrt_profile.argtypes = [ctypes.c_char_p]
    lib.axon_stop_nrt_profile.restype = ctypes.c_int64

    @contextlib.contextmanager
    def _hook(output_dir, device_ids):
        import jax
        jax.devices()  # force PJRT init so the .so's client exists
        if device_ids:
            ids = (ctypes.c_int64 * len(device_ids))(*device_ids)
            rc = lib.axon_start_nrt_profile(ids, len(device_ids))
        else:
            rc = lib.axon_start_nrt_profile(None, 0)
        if rc != 0:
            raise RuntimeError(f"axon_start_nrt_profile rc={rc}")
        try:
            yield
        finally:
            n = lib.axon_stop_nrt_profile(str(output_dir).encode())
            print(f"ntff profile: {n} file(s) -> {output_dir}", file=sys.stderr)

    import antenv
    mod = types.ModuleType("antenv.axon_hooks")
    mod.get_axon_ntff_profile_hook = lambda: _hook
    mod.set_axon_ntff_profile_hook = lambda h: None
    sys.modules["antenv.axon_hooks"] = mod
    antenv.axon_hooks = mod

B, S, D, F, E = 4, 2048, 1024, 4096, 8
T = B * S
TOP_K = 2
NCORES = 8
P = 128
ND, NF = D // P, F // P  # 8, 32
NFO = F // 512           # 8 (512-wide stripes of F)

# test.py pokes these for profiling info
LAST_RESULT = None

_cache = {}


def _chunk_list(C):
    """Token chunks (PSUM free-dim <= 512, multiples of 128).

    Chunks below 256 run LDWEIGHTS-bound on the PE (weight load ~60ns
    vs a 53ns N=128 matmul), so a short tail is split off the previous
    512 chunk into two >=256 pieces instead.
    """
    chunks = [512] * (C // 512)
    rem = C % 512
    if rem:
        if rem < 256 and chunks:
            total = 512 + rem
            a = ((total // 2 + 127) // 128) * 128
            chunks[-1] = a
            chunks.append(total - a)
        else:
            chunks.append(rem)
    return chunks


def _build_bass(C):
    dt = mybir.dt
    io_dt = dt.float16
    nc = bacc.Bacc("TRN2", target_bir_lowering=False, debug=False)

    xT = nc.dram_tensor("xT", [D, C], io_dt, kind="ExternalInput")
    # W_in host-packed stripe-major [p][fo][dn][512]: each 512-wide
    # F-stripe is one contiguous 8KB-per-partition DMA.
    win = nc.dram_tensor("win", [P, NFO * ND * 512], io_dt, kind="ExternalInput")
    wout = nc.dram_tensor("wout", [F, D], io_dt, kind="ExternalInput")
    # b_in/b_out host-packed to [partition, idx] (contiguous rows; the
    # naive (fo fi) gather is 4096 4-byte descriptors on the SWDGE).
    bin_ = nc.dram_tensor("bin", [P, NF], dt.float32, kind="ExternalInput")
    bout = nc.dram_tensor("bout", [P, ND], dt.float32, kind="ExternalInput")
    wcomb = nc.dram_tensor("wcomb", [P, C], dt.float32, kind="ExternalInput")
    yT = nc.dram_tensor("yT", [D, C], dt.float32, kind="ExternalOutput")

    xT_r = xT.ap().rearrange("(dn p) c -> p dn c", p=P)
    win_r = win.ap().rearrange("p (fo dn f) -> p fo dn f", fo=NFO, f=512)
    wout_r = wout.ap().rearrange("(fn p) d -> p fn d", p=P)
    yT_r = yT.ap().rearrange("(dn p) c -> p dn c", p=P)

    chunks = _chunk_list(C)
    ck0 = chunks[0]

    with tile.TileContext(nc) as tc, ExitStack() as ctx:
        consts = ctx.enter_context(tc.tile_pool(name="consts", bufs=1))
        xpool = ctx.enter_context(tc.tile_pool(name="x", bufs=2))
        wrespool = ctx.enter_context(tc.tile_pool(name="wres", bufs=1))
        woutpool = ctx.enter_context(tc.tile_pool(name="wout", bufs=1))
        hpool = ctx.enter_context(tc.tile_pool(name="h", bufs=1))
        ypool = ctx.enter_context(tc.tile_pool(name="y", bufs=3))
        psum_h = ctx.enter_context(tc.tile_pool(name="ph", bufs=4, space="PSUM"))
        psum_y = ctx.enter_context(tc.tile_pool(name="py", bufs=2, space="PSUM"))

        # critical path for the very first matmuls: x chunk 0 + W_in
        # stripe 0 split into quarter-DMAs PAIR-INTERLEAVED across the
        # two HWDGE queues, so complete (win0, x0) k-slice pairs land
        # progressively: dn 0-1 and 4-5 usable ~12us, dn 2-3 and 6-7
        # ~16us (vs ~20us for monolithic halves). Chunk 0's first
        # accumulation group consumes dn in that arrival order. The
        # framework tracks slice-level DMA deps precisely (measured),
        # and 4 triggers/queue exactly fills the DMA ring, so nothing
        # blocks. (The Act queue carries nothing else until the y
        # writes, so its triggers sit safely ahead of the gelus.)
        x0_t = xpool.tile([P, ND, ck0], io_dt, tag="x")
        win_t = [wrespool.tile([P, ND, 512], io_dt, name=f"wfo{fo}")
                 for fo in range(NFO)]
        nc.sync.dma_start(win_t[0][:, 0:2, :], win_r[:, 0, 0:2, :])
        nc.scalar.dma_start(x0_t[:, 4:6, :], xT_r[:, 4:6, 0:ck0])
        nc.sync.dma_start(x0_t[:, 0:2, :], xT_r[:, 0:2, 0:ck0])
        nc.scalar.dma_start(win_t[0][:, 4:6, :], win_r[:, 0, 4:6, :])
        nc.sync.dma_start(win_t[0][:, 2:4, :], win_r[:, 0, 2:4, :])
        nc.scalar.dma_start(x0_t[:, 6:8, :], xT_r[:, 6:8, 0:ck0])
        nc.sync.dma_start(x0_t[:, 2:4, :], xT_r[:, 2:4, 0:ck0])
        nc.scalar.dma_start(win_t[0][:, 6:8, :], win_r[:, 0, 6:8, :])
        dn_order0 = [0, 1, 4, 5, 2, 3, 6, 7]

        # b_in is needed by the first gelu; host-packed rows on the
        # SWDGE queue land ~13us, the first gelu runs ~27us.
        bin_t = consts.tile([P, NF], dt.float32)
        nc.gpsimd.dma_start(bin_t[:], bin_.ap())

        # PE HAM warm-up: junk matmuls on a scratch tile while the
        # x0/win0 DMAs are in flight, so the HAM activity ramp starts
        # early and the PE never idles >~2-4us (which would re-trigger
        # the clamp). The memset runs on the DVE (live ~3us vs gpsimd's
        # ~10us); the junk burn spans ~7.6-10.5us, just ahead of the
        # first real matmul at ~12us.
        wu_t = consts.tile([P, P], io_dt)
        nc.vector.memset(wu_t[:], 0.0)
        wu_ps = ctx.enter_context(tc.tile_pool(name="wups", bufs=1, space="PSUM"))
        wu_p = wu_ps.tile([P, 64], dt.float32)
        for _ in range(60):
            nc.tensor.matmul(wu_p[:], wu_t[:], wu_t[:, :64], start=True, stop=True)

        bout_t = consts.tile([P, ND], dt.float32)
        w_t = consts.tile([P, C], dt.float32)
        wout_tiles = [woutpool.tile([P, 4, D], io_dt, name=f"wout{fo}")
                      for fo in range(NFO)]

        # ---- main loop ------------------------------------------------
        off = 0
        for ci, ck in enumerate(chunks):
            csl = slice(off, off + ck)
            last = ci == len(chunks) - 1
            if ci == 0:
                x_t = x0_t
            else:
                x_t = xpool.tile([P, ND, ck], io_dt, tag="x")
                nc.sync.dma_start(x_t[:], xT_r[:, :, csl])

            # ---- phase A: h = gelu(W_in^T @ x + b_in), laid out [f, tok]
            h_t = hpool.tile([P, NF, ck], io_dt, tag="h")
            for fo in range(NFO):
                if ci == 0 and fo > 0:
                    # resident W_in stripe fo, emitted just ahead of its
                    # consumption slot (the 4-deep DMA ring gives ~3-4
                    # stripes of prefetch without reordering hazards).
                    nc.sync.dma_start(win_t[fo][:], win_r[:, fo, :, :])
                for j in range(4):
                    fc = fo * 4 + j
                    ph = psum_h.tile([P, ck], dt.float32, tag="ph")
                    # chunk 0's first group consumes k-slices in DMA
                    # arrival order (accumulation order is free)
                    dns = dn_order0 if (ci == 0 and fc == 0) else range(ND)
                    for i, dn in enumerate(dns):
                        nc.tensor.matmul(
                            ph[:],
                            win_t[fo][:, dn, j * P:(j + 1) * P],
                            x_t[:, dn, :],
                            start=(i == 0),
                            stop=(i == ND - 1),
                        )
                    nc.scalar.activation(
                        h_t[:, fc, :], ph[:],
                        mybir.ActivationFunctionType.Gelu,
                        bias=bin_t[:, fc:fc + 1],
                    )
                if ci == 0:
                    # resident W_out stripe fo streams on the SCALAR
                    # queue (one trigger per ~7us of gelus — never >2
                    # outstanding, so no ring wait ever blocks a gelu),
                    # leaving the whole sync queue to the just-in-time
                    # W_in stripes; all of W_out lands before phase B.
                    nc.scalar.dma_start(
                        wout_tiles[fo][:], wout_r[:, fo * 4:(fo + 1) * 4, :])
                    if fo == 3:
                        nc.sync.dma_start(bout_t[:], bout.ap())
                    elif fo == 5:
                        nc.sync.dma_start(w_t[:], wcomb.ap())

            # ---- phase B: y = w * (W_out^T @ h + b_out), laid out [d, tok]
            for dn in range(ND):
                py = psum_y.tile([P, ck], dt.float32, tag="py")
                for fc in range(NF):
                    nc.tensor.matmul(
                        py[:],
                        wout_tiles[fc // 4][:, fc % 4, dn * P:(dn + 1) * P],
                        h_t[:, fc, :],
                        start=(fc == 0),
                        stop=(fc == NF - 1),
                    )
                y_t = ypool.tile([P, ck], dt.float32, tag="y")
                # one DVE op: (psum + b_out) * w — keeps ScalarE on
                # gelu only (no ACT table switching per chunk)
                nc.vector.scalar_tensor_tensor(
                    y_t[:], py[:], bout_t[:, dn:dn + 1], w_t[:, csl],
                    op0=mybir.AluOpType.add, op1=mybir.AluOpType.mult,
                )
                # steady state keeps y on the scalar queue (it carries
                # nothing else); the final chunk alternates queues so
                # the output drain at end-of-kernel runs in parallel.
                if last:
                    eng = (nc.scalar, nc.sync)[dn % 2]
                else:
                    eng = nc.scalar
                eng.dma_start(yT_r[:, dn, csl], y_t[:])
            off += ck

    nc.compile()
    return nc


def _get_nc(C):
    if C not in _cache:
        _cache[C] = _build_bass(C)
    return _cache[C]


def _route(x, W_router):
    """Host-side router: top-2 selection + renormalized weights (fp64).

    Matches jax.lax.top_k on softmax(logits): softmax is monotone so
    top-2 of logits is identical, with ties broken toward lower index
    (argsort stable on -logits).
    """
    lg = x.astype(np.float64) @ W_router.T.astype(np.float64)
    top2 = np.argsort(-lg, axis=1, kind="stable")[:, :TOP_K]
    l1 = np.take_along_axis(lg, top2[:, 0:1], 1)
    l2 = np.take_along_axis(lg, top2[:, 1:2], 1)
    e2 = np.exp(l2 - l1)
    w1 = (1.0 / (1.0 + e2)).astype(np.float32)
    w2 = (e2 / (1.0 + e2)).astype(np.float32)
    return top2, np.concatenate([w1, w2], axis=1)


def _pack_win(W):
    """[D, F] fp16 -> [P, NFO*ND*512] laid out [p][fo][dn][512]: each
    512-wide F-stripe becomes one contiguous 8KB-per-partition block."""
    A = W.reshape(ND, P, NFO, 512).transpose(1, 2, 0, 3)  # [p, fo, dn, f]
    return np.ascontiguousarray(A).reshape(P, NFO * ND * 512)


def kernel(residual, W_router, W_in, b_in, W_out, b_out):
    global LAST_RESULT

    x = np.ascontiguousarray(np.asarray(residual, dtype=np.float32).reshape(T, D))
    W_in = np.asarray(W_in, dtype=np.float32)
    W_out = np.asarray(W_out, dtype=np.float32)
    b_in = np.asarray(b_in, dtype=np.float32)
    b_out = np.asarray(b_out, dtype=np.float32)

    top2, wts = _route(x, np.asarray(W_router, dtype=np.float32))

    idxs, ws = [], []
    for e in range(E):
        sel0 = top2[:, 0] == e
        sel1 = top2[:, 1] == e
        idx = np.concatenate([np.where(sel0)[0], np.where(sel1)[0]])
        w = np.concatenate([wts[sel0, 0], wts[sel1, 1]])
        idxs.append(idx)
        ws.append(w)

    C = max(len(i) for i in idxs)
    C = ((C + P - 1) // P) * P
    nc = _get_nc(C)

    xt = np.ascontiguousarray(x.T)  # [D, T]
    in_maps = []
    for e in range(E):
        cnt = len(idxs[e])
        xT_e = np.zeros((D, C), dtype=np.float16)
        xT_e[:, :cnt] = xt[:, idxs[e]]
        wc_e = np.zeros((P, C), dtype=np.float32)
        wc_e[:, :cnt] = ws[e][None, :]
        in_maps.append({
            "xT": xT_e,
            "win": _pack_win(np.asarray(W_in[e], dtype=np.float16)),
            "wout": np.ascontiguousarray(W_out[e], dtype=np.float16),
            "bin": np.ascontiguousarray(b_in[e].reshape(NF, P).T),
            "bout": np.ascontiguousarray(b_out[e].reshape(ND, P).T),
            "wcomb": wc_e,
        })

    if os.environ.get("BASS_TRACE"):
        _install_ntff_hook()
    LAST_RESULT = run_bass_kernel_spmd(nc, in_maps, list(range(NCORES)))

    y = np.zeros((T, D), dtype=np.float32)
    for e in range(E):
        cnt = len(idxs[e])
        y[idxs[e]] += LAST_RESULT.results[e]["yT"][:, :cnt].T
    return y.reshape(B, S, D)
